# revision 1
# baseline (speedup 1.0000x reference)
"""Trainium2 Bass kernel for nn_CausalSelfAttention (B=4, L=2048, D=1024, H=16).

Sharding: 2 heads per core (tensor parallel) x 8 cores, all batches on every
core.  Each core computes qkv for its 2 heads over all tokens (reading full x),
runs causal attention, and produces a partial projection output
yT_c = proj_w[rows_c].T @ O_c^T  of shape [D, B*L].  The host sums the 8
partials, transposes, and adds proj_b.

Device pipeline per core (all matmuls fp32r: 1 cycle/row at N>=256):
  x [tok,d] --PE transpose--> xT tiles [d,tok]
  qkvT [feat,tok] = w_tile.T @ xT (+bias via K=1 matmul)
  RoPE: rot(q) via signed-permutation matmul on PE, then
        qT_roped = qT*cos + rot(qT)*sin  (3 full-height DVE ops)
  V: PE-transpose back to tok-major, stored as [V|1] tiles
  S^T[k,q] = KT_slice.T @ QT_slice (heads interleaved -> PE row-group overlap)
  P^T = exp(S^T/8) (ACT, causal via affine_select on diagonal stripes)
  O^T[hd+1,q] += [V|1].T @ P^T
  normalize via reciprocal_approx_fast + gpsimd partition_broadcast + DVE mul
  yT += pw_h.T @ OT_h (two K=64 matmuls per tile)
The normalize+projection of iteration i-1 is emitted after the qkv phase of
iteration i so the PE never idles on the normalization chain (HAM stays warm).
"""

import numpy as np

import concourse.bass as bass  # noqa: F401
import concourse.tile as tile
from concourse import mybir, bacc
from concourse import bass_utils
from concourse.masks import make_identity

f32 = mybir.dt.float32
f32r = mybir.dt.float32r
AL = mybir.AluOpType
AF = mybir.ActivationFunctionType


class _Bacc(bacc.Bacc):
    """Bacc that pins all activations to the one table set holding both
    ln and exp (plus copy/identity), so the per-iteration Ln<->Exp pair
    doesn't thrash ACT_TABLE_LOADs (~2.7us each)."""

    def insert_act_table_loads(self):
        import bass_rust as _bass_rust
        from concourse.hw_specs import get_activation_tables

        has_activation = any(
            isinstance(i, mybir.InstActivation)
            for bb in self.main_func.blocks
            for i in bb.instructions
        )
        if not has_activation:
            return
        # act_func_set_id is positional: keep the full list order, but empty
        # every other set so the chooser can only pick the combined one.
        tables = [
            (k, v if k == "natural_log_exp_and_others" else set())
            for k, v in get_activation_tables(self.m.arch).items()
        ]
        _bass_rust.insert_act_table_loads(self, tables)

HIDDEN = 1024
HEADS = 16
HD = 64
ROPE_BASE = 10000.0
N_CORES = 8
H2 = 2           # heads per core
F = 3 * H2 * HD  # 384 qkv feature columns per core
QCH = 512        # token chunk = attention q granule
DT = HIDDEN // 128  # 8 d tiles


def build_program(NB, T):
    """Build the per-core Bass program: NB batches of T tokens each."""
    assert T % QCH == 0
    NTOK = NB * T
    NKT = T // 128  # k tiles per batch
    nc = _Bacc("TRN2", target_bir_lowering=False, debug=False,
               num_devices=N_CORES)

    x = nc.dram_tensor("x", [NTOK, HIDDEN], f32r, kind="ExternalInput").ap()
    w = nc.dram_tensor("w", [HIDDEN, F], f32r, kind="ExternalInput").ap()
    brow = nc.dram_tensor("brow", [1, F], f32r, kind="ExternalInput").ap()
    psgn = nc.dram_tensor("psgn", [128, 128], f32r, kind="ExternalInput").ap()
    pw = nc.dram_tensor("pw", [128, HIDDEN], f32r, kind="ExternalInput").ap()
    cos_t = nc.dram_tensor("cos_t", [128, T], f32, kind="ExternalInput").ap()
    sin_t = nc.dram_tensor("sin_t", [128, T], f32, kind="ExternalInput").ap()
    yT = nc.dram_tensor("yT", [HIDDEN, NTOK], f32, kind="ExternalOutput").ap()

    with tile.TileContext(nc) as tc:
        with tc.tile_pool(name="const", bufs=1) as constp, \
             tc.tile_pool(name="resident", bufs=1) as resp, \
             tc.tile_pool(name="xload", bufs=6) as xp, \
             tc.tile_pool(name="xt", bufs=12) as xtp, \
             tc.tile_pool(name="rope", bufs=3) as ropep, \
             tc.tile_pool(name="qtcur", bufs=2) as qtp, \
             tc.tile_pool(name="pt", bufs=4) as ptp, \
             tc.tile_pool(name="ot", bufs=3) as otp, \
             tc.tile_pool(name="ysb", bufs=3) as yp, \
             tc.tile_pool(name="small", bufs=4) as smp, \
             tc.tile_pool(name="ps_s", bufs=2, space="PSUM") as ps_s_p, \
             tc.tile_pool(name="ps_o", bufs=2, space="PSUM") as ps_o_p, \
             tc.tile_pool(name="ps_m", bufs=2, space="PSUM") as ps_m_p:

            # ---- constants / residents ----
            ident_f = constp.tile([128, 128], f32)
            make_identity(nc, ident_f[:])
            ident = constp.tile([128, 128], f32r)
            nc.vector.tensor_copy(ident[:], ident_f[:])
            # w tiles: per d-tile, F columns
            w_sb = constp.tile([128, DT * F], f32r)
            for dt in range(DT):
                nc.sync.dma_start(w_sb[:, dt * F:(dt + 1) * F],
                                  w[dt * 128:(dt + 1) * 128, :])
            brow_sb = constp.tile([1, F], f32r)
            nc.sync.dma_start(brow_sb[:], brow[:])
            psgn_sb = constp.tile([128, 128], f32r)
            nc.sync.dma_start(psgn_sb[:], psgn[:])
            ones_f = constp.tile([128, 512], f32)
            nc.gpsimd.memset(ones_f[:], 1.0)
            ones_row = constp.tile([1, 512], f32r)
            nc.vector.tensor_copy(ones_row[:], ones_f[0:1, :])
            pw_sb = constp.tile([128, HIDDEN], f32r)
            nc.sync.dma_start(pw_sb[:], pw[:])
            cos_sb = constp.tile([128, T], f32)
            nc.sync.dma_start(cos_sb[:], cos_t[:])
            sin_sb = constp.tile([128, T], f32)
            nc.sync.dma_start(sin_sb[:], sin_t[:])

            KT_res = resp.tile([128, T], f32r)
            V_res = resp.tile([128, NKT * 130], f32r)
            v4 = V_res[:].rearrange("p (kt h c) -> p kt h c", kt=NKT, h=2)
            nc.gpsimd.tensor_copy(
                v4[:, :, :, 64],
                ones_f[:, :2 * NKT].rearrange("p (kt h) -> p kt h", kt=NKT))

            def norm_part(st):
                O, t0v = st
                ot_full = otp.tile([128, 512], f32r, tag="ot", name="ot_full")
                for h in range(2):
                    # 1/rowsum = exp(-ln(rowsum)) on ACT (same table set as
                    # the attention Exp; DVE reciprocal is 3.3us and would
                    # stall the pipeline)
                    lnv = smp.tile([1, 512], f32, tag="ln", name="lnv")
                    nc.scalar.activation(lnv[:], O[h][64:65, :], AF.Ln)
                    rs_sb = smp.tile([1, 512], f32, tag="rs", name="rs")
                    nc.scalar.activation(rs_sb[:], lnv[:], AF.Exp,
                                         bias=0.0, scale=-1.0)
                    rsb = smp.tile([64, 512], f32, tag="rsb", name="rsb")
                    nc.gpsimd.partition_broadcast(rsb[:], rs_sb[:])
                    nc.vector.tensor_tensor(ot_full[64 * h:64 * h + 64, :],
                                            O[h][0:64, :], rsb[:], AL.mult)
                return ot_full

            def proj_part(st, ot_full):
                O, t0v = st
                for oi in range(8):
                    ps_y = ps_m_p.tile([128, 512], f32, tag="m", name="ps_y")
                    nc.tensor.matmul(
                        ps_y[:], pw_sb[:, oi * 128:(oi + 1) * 128],
                        ot_full[:], start=True, stop=True)
                    ysb = yp.tile([128, 512], f32, tag="y", name="ysb")
                    if oi % 2 == 0:
                        nc.vector.tensor_copy(ysb[:], ps_y[:])
                    else:
                        nc.scalar.copy(ysb[:], ps_y[:])
                    nc.sync.dma_start(
                        yT[oi * 128:(oi + 1) * 128, t0v:t0v + 512], ysb[:])

            prev = None
            for b in range(NB):
                for qc in range(T // QCH):
                    Q0 = qc * QCH
                    t0 = b * T + Q0
                    prev_ots = norm_part(prev) if prev is not None else None
                    # ---------- qkv phase for tokens [t0, t0+512) ----------
                    xa = [xp.tile([128, HIDDEN], f32r, tag="x", name=f"xa{tt}")
                          for tt in range(4)]
                    for tt in range(4):
                        nc.sync.dma_start(
                            xa[tt][:], x[t0 + tt * 128: t0 + (tt + 1) * 128, :])
                    xt_sb = [xtp.tile([128, QCH], f32r, tag="xt",
                                      name=f"xt{dt}") for dt in range(DT)]
                    for dt in range(DT):
                        ps_xt = ps_m_p.tile([128, QCH], f32r, tag="m",
                                            name="ps_xt")
                        for tt in range(4):
                            nc.tensor.transpose(
                                ps_xt[:, tt * 128:(tt + 1) * 128],
                                xa[tt][:, dt * 128:(dt + 1) * 128], ident[:])
                        nc.vector.tensor_copy(xt_sb[dt][:], ps_xt[:])
                    QT_cur = qtp.tile([128, QCH], f32r, tag="qt", name="QT")
                    for f in range(3):  # 0=q, 1=k, 2=v
                        ps_f = ps_m_p.tile([128, QCH], f32, tag="m",
                                           name="ps_f")
                        for dt in range(DT):
                            nc.tensor.matmul(
                                ps_f[:],
                                w_sb[:, dt * F + f * 128:dt * F + (f + 1) * 128],
                                xt_sb[dt][:], start=(dt == 0), stop=False)
                        nc.tensor.matmul(
                            ps_f[:], brow_sb[:, f * 128:(f + 1) * 128],
                            ones_row[:], start=False, stop=True)
                        raw = ropep.tile([128, QCH], f32r, tag="raw",
                                         name="raw")
                        nc.scalar.copy(raw[:], ps_f[:])
                        if f < 2:
                            ps_rot = ps_m_p.tile([128, QCH], f32, tag="m",
                                                 name="ps_rot")
                            nc.tensor.matmul(ps_rot[:], psgn_sb[:], raw[:],
                                             start=True, stop=True)
                            t1 = ropep.tile([128, QCH], f32, tag="t1",
                                            name="t1")
                            nc.vector.tensor_tensor(
                                t1[:], raw[:], cos_sb[:, Q0:Q0 + QCH], AL.mult)
                            t2 = ropep.tile([128, QCH], f32, tag="t2",
                                            name="t2")
                            nc.vector.tensor_tensor(
                                t2[:], ps_rot[:], sin_sb[:, Q0:Q0 + QCH],
                                AL.mult)
                            dst = (QT_cur[:] if f == 0
                                   else KT_res[:, Q0:Q0 + QCH])
                            nc.vector.tensor_tensor(dst, t1[:], t2[:], AL.add)
                        else:
                            for tt in range(4):
                                ps_vt = ps_m_p.tile([128, 128], f32r, tag="m",
                                                    name="ps_vt")
                                nc.tensor.transpose(
                                    ps_vt[:],
                                    raw[:, tt * 128:(tt + 1) * 128], ident[:])
                                kt = Q0 // 128 + tt
                                nc.vector.tensor_copy(
                                    v4[:, kt, :, 0:64],
                                    ps_vt[:].rearrange("p (h j) -> p h j", h=2))
                    # ---------- deferred projection ----------
                    if prev is not None:
                        proj_part(prev, prev_ots)
                    # ---------- attention for (b, qc) ----------
                    nkb = (Q0 + QCH) // 128
                    O = [ps_o_p.tile([65, 512], f32, tag="o", name=f"O{h}")
                         for h in range(2)]
                    for kb in range(nkb):
                        qstart = max(0, 128 * kb - Q0)
                        # both heads' scores in one 2-bank psum tile; single
                        # exp over a [p, 2, width] AP halves ACT op overhead
                        ps_sc = ps_s_p.tile([128, 2 * QCH], f32, tag="s",
                                            name="ps_sc")
                        for h in range(2):
                            hp = slice(64 * h, 64 * h + 64)
                            nc.tensor.matmul(
                                ps_sc[:, QCH * h + qstart:QCH * h + QCH],
                                KT_res[hp, kb * 128:(kb + 1) * 128],
                                QT_cur[hp, qstart:QCH],
                                start=True, stop=True)
                        pt = ptp.tile([128, 2 * QCH], f32r, tag="pt",
                                      name="pt")
                        sc4 = ps_sc[:].rearrange("p (h q) -> p h q", h=2)
                        pt4 = pt[:].rearrange("p (h q) -> p h q", h=2)
                        nc.scalar.activation(pt4[:, :, qstart:QCH],
                                             sc4[:, :, qstart:QCH],
                                             AF.Exp, bias=0.0, scale=0.125)
                        if 128 * kb >= Q0:
                            ds = 128 * kb - Q0
                            for h in range(2):
                                nc.gpsimd.affine_select(
                                    out=pt[:, QCH * h + ds:QCH * h + ds + 128],
                                    in_=pt[:, QCH * h + ds:QCH * h + ds + 128],
                                    pattern=[[1, 128]], compare_op=AL.is_ge,
                                    fill=0.0, base=0, channel_multiplier=-1)
                        for h in range(2):
                            nc.tensor.matmul(
                                O[h][:, qstart:QCH],
                                V_res[:, 130 * kb + 65 * h:
                                      130 * kb + 65 * h + 65],
                                pt[:, QCH * h + qstart:QCH * h + QCH],
                                start=(kb == 0), stop=(kb == nkb - 1))
                    prev = (O, t0)
            proj_part(prev, norm_part(prev))
    nc.compile()
    return nc


# ---------------------------------------------------------------- host side

def _rope_tables(T):
    inv_freq = 1.0 / (ROPE_BASE ** (np.arange(0, HD, 2, dtype=np.float64) / HD))
    pos = np.arange(T, dtype=np.float64)
    ang = np.outer(pos, inv_freq)          # [T, 32]
    cos = np.cos(ang).astype(np.float32)   # [T, 32]
    sin = np.sin(ang).astype(np.float32)
    jm32 = np.arange(128) % 32
    # feat-major: row r (feature), col t (within-batch position)
    cos_t = np.ascontiguousarray(cos[:, jm32].T)   # [128, T]
    sin_t = np.ascontiguousarray(sin[:, jm32].T)
    return cos_t, sin_t


def _psgn():
    p = np.zeros((HD, HD), np.float32)
    for i in range(32):
        p[i + 32, i] = -1.0   # out dim i (<32) = -in dim i+32
        p[i, i + 32] = 1.0    # out dim i+32   = +in dim i
    pf = np.zeros((128, 128), np.float32)
    pf[0:64, 0:64] = p        # head A block
    pf[64:128, 64:128] = p    # head B block
    return np.ascontiguousarray(pf)


def make_core_inputs(x, qkv_w, qkv_b, proj_w, NB, T):
    NTOK = NB * T
    xf = np.ascontiguousarray(
        np.asarray(x).reshape(NTOK, HIDDEN).astype(np.float32))
    cos_t, sin_t = _rope_tables(T)
    psgn = _psgn()
    in_maps = []
    for c in range(N_CORES):
        col = HD * H2 * c
        wq = qkv_w[:, col:col + 128]
        wk = qkv_w[:, HIDDEN + col:HIDDEN + col + 128]
        wv = qkv_w[:, 2 * HIDDEN + col:2 * HIDDEN + col + 128]
        wc = np.ascontiguousarray(
            np.concatenate([wq, wk, wv], axis=1).astype(np.float32))
        bq = qkv_b[col:col + 128]
        bk = qkv_b[HIDDEN + col:HIDDEN + col + 128]
        bv = qkv_b[2 * HIDDEN + col:2 * HIDDEN + col + 128]
        browc = np.ascontiguousarray(
            np.concatenate([bq, bk, bv])[None, :].astype(np.float32))
        pwc = np.ascontiguousarray(
            proj_w[col:col + 128, :].astype(np.float32))
        in_maps.append({
            "x": xf, "w": wc, "brow": browc, "psgn": psgn, "pw": pwc,
            "cos_t": cos_t, "sin_t": sin_t,
        })
    return in_maps


_PROGRAM_CACHE = {}


def _get_program(NB, T):
    key = (NB, T)
    if key not in _PROGRAM_CACHE:
        _PROGRAM_CACHE[key] = build_program(NB, T)
    return _PROGRAM_CACHE[key]


def run(x, qkv_w, qkv_b, proj_w, proj_b, NB, T, trace=False):
    nc = _get_program(NB, T)
    in_maps = make_core_inputs(x, qkv_w, qkv_b, proj_w, NB, T)
    res = bass_utils.run_bass_kernel_spmd(
        nc, in_maps, core_ids=list(range(N_CORES)), trace=trace)
    acc = res.results[0]["yT"].astype(np.float32).copy()
    for c in range(1, N_CORES):
        acc += res.results[c]["yT"]
    out = (acc.T.reshape(NB, T, HIDDEN)
           + np.asarray(proj_b)[None, None, :].astype(np.float32))
    return out, res


def kernel(x, qkv_w, qkv_b, proj_w, proj_b):
    x = np.asarray(x)
    B, L, D = x.shape
    out, _ = run(x, np.asarray(qkv_w), np.asarray(qkv_b),
                 np.asarray(proj_w), np.asarray(proj_b), NB=B, T=L)
    return out.astype(np.float32)



# revision 6
# speedup vs baseline: 1.1448x; 1.1448x over previous
"""Trainium2 Bass kernel for nn_CausalSelfAttention (B=4, L=2048, D=1024, H=16).

Sharding: 2 heads per core (tensor parallel) x 8 cores, all batches on every
core.  Each core computes qkv for its 2 heads over all tokens, runs causal
attention, and produces a partial projection output y_c [NTOK, D] (its heads'
contribution).  The host sums the 8 partials and adds proj_b.

Key layout/перф choices vs the v0 kernel:
  * x is transposed on the HOST -> the kernel DMAs xT [d, tok] tiles straight
    to SBUF; no PE transposes / DVE casts for x at all.
  * qkv bias is added by the DVE PSUM->SBUF copy (tensor_scalar_add with a
    per-partition bias AP) instead of K=1 matmuls.
  * proj emits y in [tok, feat] layout (lhsT = O^T chunk, rhs = proj_w rows)
    so the host reduction needs no transpose.
  * emission is software-pipelined: the attention kb-loop of chunk i carries
    interleaved packets of chunk i+1's qkv matmuls and chunk i-1's norm+proj,
    keeping the PE array continuously busy (the tensor engine clock ramps to
    full speed only after ~3us of uninterrupted work).
  * 1/rowsum via DVE reciprocal_approx_fast (frees ACT for the exp stream);
    PSUM->SBUF proj copies are spread over GpSimd/DVE/ACT.
"""

import numpy as np

import concourse.bass as bass  # noqa: F401
import concourse.tile as tile
from concourse import mybir, bacc
from concourse import bass_utils
from concourse.masks import make_identity

f32 = mybir.dt.float32
f32r = mybir.dt.float32r
AL = mybir.AluOpType
AF = mybir.ActivationFunctionType


class _Bacc(bacc.Bacc):
    """Pin activations to the table set holding both ln and exp so ACT never
    thrashs ACT_TABLE_LOADs (~1.3us each)."""

    def insert_act_table_loads(self):
        import bass_rust as _bass_rust
        from concourse.hw_specs import get_activation_tables

        has_activation = any(
            isinstance(i, mybir.InstActivation)
            for bb in self.main_func.blocks
            for i in bb.instructions
        )
        if not has_activation:
            return
        tables = [
            (k, v if k == "natural_log_exp_and_others" else set())
            for k, v in get_activation_tables(self.m.arch).items()
        ]
        _bass_rust.insert_act_table_loads(self, tables)


HIDDEN = 1024
HEADS = 16
HD = 64
ROPE_BASE = 10000.0
N_CORES = 8
H2 = 2           # heads per core
F = 3 * H2 * HD  # 384 qkv feature columns per core
QCH = 512        # token chunk = attention q granule
DT = HIDDEN // 128  # 8 d tiles


def build_program(NB, T):
    """Build the per-core Bass program: NB batches of T tokens each."""
    assert T % QCH == 0
    NTOK = NB * T
    NKT = T // 128   # k tiles per batch
    NCH = T // QCH   # q chunks per batch
    NITER = NB * NCH
    nc = _Bacc("TRN2", target_bir_lowering=False, debug=False,
               num_devices=N_CORES)

    xT = nc.dram_tensor("xT", [HIDDEN, NTOK], f32r, kind="ExternalInput").ap()
    w = nc.dram_tensor("w", [HIDDEN, F], f32r, kind="ExternalInput").ap()
    bcol = nc.dram_tensor("bcol", [128, 3], f32, kind="ExternalInput").ap()
    psgn = nc.dram_tensor("psgn", [128, 128], f32r, kind="ExternalInput").ap()
    pw = nc.dram_tensor("pw", [128, HIDDEN], f32r, kind="ExternalInput").ap()
    cos_t = nc.dram_tensor("cos_t", [128, T], f32, kind="ExternalInput").ap()
    sin_t = nc.dram_tensor("sin_t", [128, T], f32, kind="ExternalInput").ap()
    y = nc.dram_tensor("y", [NTOK, HIDDEN], f32, kind="ExternalOutput").ap()

    with tile.TileContext(nc) as tc:
        with tc.tile_pool(name="const", bufs=1) as constp, \
             tc.tile_pool(name="resident", bufs=1) as resp, \
             tc.tile_pool(name="xload", bufs=3) as xp, \
             tc.tile_pool(name="rope", bufs=3) as ropep, \
             tc.tile_pool(name="qtcur", bufs=2) as qtp, \
             tc.tile_pool(name="pt", bufs=3) as ptp, \
             tc.tile_pool(name="ot", bufs=2) as otp, \
             tc.tile_pool(name="ysb", bufs=3) as yp, \
             tc.tile_pool(name="small", bufs=4) as smp, \
             tc.tile_pool(name="ps_s", bufs=2, space="PSUM") as ps_s_p, \
             tc.tile_pool(name="ps_o", bufs=2, space="PSUM") as ps_o_p, \
             tc.tile_pool(name="ps_m", bufs=2, space="PSUM") as ps_m_p:

            # ---- constants / residents ----
            ident_f = constp.tile([128, 128], f32)
            make_identity(nc, ident_f[:])
            ident = constp.tile([128, 128], f32r)
            nc.vector.tensor_copy(ident[:], ident_f[:])
            w_sb = constp.tile([128, DT * F], f32r)
            for dt in range(DT):
                nc.sync.dma_start(w_sb[:, dt * F:(dt + 1) * F],
                                  w[dt * 128:(dt + 1) * 128, :])
            bcol_sb = constp.tile([128, 3], f32)
            nc.sync.dma_start(bcol_sb[:], bcol[:])
            psgn_sb = constp.tile([128, 128], f32r)
            nc.sync.dma_start(psgn_sb[:], psgn[:])
            ones_f = constp.tile([128, 2 * NKT], f32)
            nc.gpsimd.memset(ones_f[:], 1.0)
            pw_sb = constp.tile([128, HIDDEN], f32r)
            nc.sync.dma_start(pw_sb[:], pw[:])
            cos_sb = constp.tile([128, T], f32)
            nc.sync.dma_start(cos_sb[:], cos_t[:])
            sin_sb = constp.tile([128, T], f32)
            nc.sync.dma_start(sin_sb[:], sin_t[:])

            # residents double-buffered by batch parity (avoids WAR stalls
            # when chunk i+1 of batch b+1 overwrites KT/V still being read
            # by the attention of batch b's last chunk)
            KT_res = resp.tile([128, 2 * T], f32r)
            V_res = resp.tile([128, 2 * NKT * 130], f32r)
            v5 = V_res[:].rearrange("p (r kt h c) -> p r kt h c", r=2, kt=NKT,
                                    h=2)
            for par in range(2):
                nc.gpsimd.tensor_copy(
                    v5[:, par, :, :, 64],
                    ones_f[:].rearrange("p (kt h) -> p kt h", kt=NKT))

            # per-iteration state passed between pipeline stages
            qt_tiles = {}    # it -> QT tile
            xa_tiles = {}    # it -> x chunk tile
            o_tiles = {}     # it -> (O[h] psum tiles, t0)

            def emit_xload(it):
                b, qc = divmod(it, NCH)
                t0 = b * T + qc * QCH
                xa = xp.tile([128, DT * QCH], f32r, tag="xa", name="xa")
                src = xT[:].rearrange("(dt p) n -> p dt n", p=128)
                nc.sync.dma_start(
                    xa[:].rearrange("p (dt n) -> p dt n", dt=DT),
                    src[:, :, t0:t0 + QCH])
                xa_tiles[it] = xa

            def qkv_packets(it):
                """Closures emitting chunk it's qkv phase (~1 PE-slot each)."""
                b, qc = divmod(it, NCH)
                Q0 = qc * QCH
                par = b % 2
                xa = xa_tiles.pop(it)
                pk = []
                QT_cur = qtp.tile([128, QCH], f32r, tag="qt", name="QT")
                qt_tiles[it] = QT_cur
                for f in range(3):  # 0=q, 1=k, 2=v
                    ps_f = ps_m_p.tile([128, QCH], f32, tag="m", name="ps_f")
                    for dt0 in range(0, DT, 2):
                        def mm2(f=f, dt0=dt0, ps_f=ps_f, xa=xa):
                            for dt in (dt0, dt0 + 1):
                                nc.tensor.matmul(
                                    ps_f[:],
                                    w_sb[:, dt * F + f * 128:
                                         dt * F + (f + 1) * 128],
                                    xa[:, dt * QCH:dt * QCH + QCH],
                                    start=(dt == 0), stop=(dt == DT - 1))
                        pk.append(mm2)
                    raw = ropep.tile([128, QCH], f32r, tag="raw", name="raw")

                    def cp(raw=raw, ps_f=ps_f, f=f):
                        nc.vector.tensor_scalar_add(
                            raw[:], ps_f[:], bcol_sb[:, f:f + 1])
                    pk.append(cp)
                    if f < 2:
                        ps_rot = ps_m_p.tile([128, QCH], f32, tag="m",
                                             name="ps_rot")

                        def rot(ps_rot=ps_rot, raw=raw):
                            nc.tensor.matmul(ps_rot[:], psgn_sb[:], raw[:],
                                             start=True, stop=True)
                        pk.append(rot)
                        t1 = ropep.tile([128, QCH], f32, tag="t1", name="t1")
                        t2 = ropep.tile([128, QCH], f32, tag="t2", name="t2")
                        dst = (QT_cur[:] if f == 0
                               else KT_res[:, par * T + Q0:par * T + Q0 + QCH])

                        def rmul(t1=t1, t2=t2, raw=raw, ps_rot=ps_rot, Q0=Q0):
                            nc.vector.tensor_tensor(
                                t1[:], raw[:], cos_sb[:, Q0:Q0 + QCH], AL.mult)
                            nc.vector.tensor_tensor(
                                t2[:], ps_rot[:], sin_sb[:, Q0:Q0 + QCH],
                                AL.mult)
                        pk.append(rmul)

                        def radd(dst=dst, t1=t1, t2=t2):
                            nc.vector.tensor_tensor(dst, t1[:], t2[:], AL.add)
                        pk.append(radd)
                    else:
                        for tt in range(4):
                            def vtr(tt=tt, raw=raw, par=par, Q0=Q0):
                                ps_vt = ps_m_p.tile([128, 128], f32r, tag="m",
                                                    name="ps_vt")
                                nc.tensor.transpose(
                                    ps_vt[:],
                                    raw[:, tt * 128:(tt + 1) * 128], ident[:])
                                kt = Q0 // 128 + tt
                                nc.vector.tensor_copy(
                                    v5[:, par, kt, :, 0:64],
                                    ps_vt[:].rearrange("p (h j) -> p h j",
                                                       h=2))
                            pk.append(vtr)
                return pk

            def attn_steps(it):
                """Per-kb closures for chunk it's attention; O accumulates in
                PSUM across all kb."""
                b, qc = divmod(it, NCH)
                Q0 = qc * QCH
                par = b % 2
                t0 = b * T + Q0
                nkb = (Q0 + QCH) // 128
                QT_cur = qt_tiles.pop(it)
                O = [ps_o_p.tile([65, QCH], f32, tag="o", name=f"O{h}")
                     for h in range(2)]
                o_tiles[it] = (O, t0)
                sc_tiles = {}
                pt_tiles = {}

                def emit_S(kb):
                    qstart = max(0, 128 * kb - Q0)
                    ps_sc = ps_s_p.tile([128, 2 * QCH], f32, tag="s",
                                        name="ps_sc")
                    sc_tiles[kb] = ps_sc
                    for h in range(2):
                        hp = slice(64 * h, 64 * h + 64)
                        nc.tensor.matmul(
                            ps_sc[:, QCH * h + qstart:QCH * h + QCH],
                            KT_res[hp, par * T + kb * 128:
                                   par * T + (kb + 1) * 128],
                            QT_cur[hp, qstart:QCH],
                            start=True, stop=True)

                def step(kb):
                    if kb == 0:
                        emit_S(0)
                    qstart = max(0, 128 * kb - Q0)
                    ps_sc = sc_tiles.pop(kb)
                    pt = ptp.tile([128, 2 * QCH], f32r, tag="pt", name="pt")
                    pt_tiles[kb] = pt
                    sc4 = ps_sc[:].rearrange("p (h q) -> p h q", h=2)
                    pt4 = pt[:].rearrange("p (h q) -> p h q", h=2)
                    nc.scalar.activation(pt4[:, :, qstart:QCH],
                                         sc4[:, :, qstart:QCH],
                                         AF.Exp, bias=0.0, scale=0.125)
                    if 128 * kb >= Q0:
                        ds = 128 * kb - Q0
                        for h in range(2):
                            nc.gpsimd.affine_select(
                                out=pt[:, QCH * h + ds:QCH * h + ds + 128],
                                in_=pt[:, QCH * h + ds:QCH * h + ds + 128],
                                pattern=[[1, 128]], compare_op=AL.is_ge,
                                fill=0.0, base=0, channel_multiplier=-1)
                    if kb + 1 < nkb:
                        emit_S(kb + 1)

                def pv(kb):
                    qstart = max(0, 128 * kb - Q0)
                    pt = pt_tiles.pop(kb)
                    for h in range(2):
                        nc.tensor.matmul(
                            O[h][:, qstart:QCH],
                            V_res[:, par * NKT * 130 + 130 * kb + 65 * h:
                                  par * NKT * 130 + 130 * kb + 65 * h + 65],
                            pt[:, QCH * h + qstart:QCH * h + QCH],
                            start=(kb == 0), stop=(kb == nkb - 1))
                return [(lambda kb=kb: step(kb), lambda kb=kb: pv(kb))
                        for kb in range(nkb)]

            # engines for the 8 proj PSUM->SBUF copies, spread for balance
            # (GpSimd cannot read PSUM, so only DVE/ACT qualify)
            _cp_eng = ["v", "a", "v", "a", "v", "a", "v", "a"]

            def post_packets(it):
                """Normalize O(it) and project to y — list of closures."""
                O, t0v = o_tiles.pop(it)
                ot_full = otp.tile([128, QCH], f32r, tag="ot", name="ot_full")

                def norm(h):
                    lnv = smp.tile([1, QCH], f32, tag="ln", name="lnv")
                    nc.scalar.activation(lnv[:], O[h][64:65, :], AF.Ln)
                    rs = smp.tile([1, QCH], f32, tag="rs", name="rs")
                    nc.scalar.activation(rs[:], lnv[:], AF.Exp,
                                         bias=0.0, scale=-1.0)
                    rsb = smp.tile([64, QCH], f32, tag="rsb", name="rsb")
                    nc.gpsimd.partition_broadcast(rsb[:], rs[:])
                    nc.vector.tensor_tensor(ot_full[64 * h:64 * h + 64, :],
                                            O[h][0:64, :], rsb[:], AL.mult)
                pk = [lambda: norm(0), lambda: norm(1)]
                for oi in range(8):
                    def proj(oi=oi, ot_full=ot_full, t0v=t0v):
                        tt, half = divmod(oi, 2)
                        ps_y = ps_m_p.tile([128, QCH], f32, tag="m",
                                           name="ps_y")
                        nc.tensor.matmul(
                            ps_y[:], ot_full[:, tt * 128:(tt + 1) * 128],
                            pw_sb[:, half * QCH:(half + 1) * QCH],
                            start=True, stop=True)
                        ysb = yp.tile([128, QCH], f32, tag="y", name="ysb")
                        if _cp_eng[oi] == "v":
                            nc.vector.tensor_copy(ysb[:], ps_y[:])
                        else:
                            nc.scalar.copy(ysb[:], ps_y[:])
                        nc.sync.dma_start(
                            y[t0v + tt * 128:t0v + (tt + 1) * 128,
                              half * QCH:(half + 1) * QCH], ysb[:])
                    pk.append(proj)
                return pk

            # ---------------- pipelined emission ----------------
            emit_xload(0)
            for p in qkv_packets(0):
                p()
            for it in range(NITER):
                if it + 1 < NITER:
                    emit_xload(it + 1)
                extras = []
                if it >= 1:
                    extras += post_packets(it - 1)
                if it + 1 < NITER:
                    extras += qkv_packets(it + 1)
                steps = attn_steps(it)
                nkb = len(steps)
                ei = 0
                for kb, (st, pv) in enumerate(steps):
                    st()
                    want = (kb + 1) * len(extras) // nkb
                    while ei < want:
                        extras[ei]()
                        ei += 1
                    pv()
                while ei < len(extras):
                    extras[ei]()
                    ei += 1
            for p in post_packets(NITER - 1):
                p()
    nc.compile()
    return nc


# ---------------------------------------------------------------- host side

def _rope_tables(T):
    inv_freq = 1.0 / (ROPE_BASE ** (np.arange(0, HD, 2, dtype=np.float64) / HD))
    pos = np.arange(T, dtype=np.float64)
    ang = np.outer(pos, inv_freq)          # [T, 32]
    cos = np.cos(ang).astype(np.float32)   # [T, 32]
    sin = np.sin(ang).astype(np.float32)
    jm32 = np.arange(128) % 32
    cos_t = np.ascontiguousarray(cos[:, jm32].T)   # [128, T]
    sin_t = np.ascontiguousarray(sin[:, jm32].T)
    return cos_t, sin_t


def _psgn():
    p = np.zeros((HD, HD), np.float32)
    for i in range(32):
        p[i + 32, i] = -1.0   # out dim i (<32) = -in dim i+32
        p[i, i + 32] = 1.0    # out dim i+32   = +in dim i
    pf = np.zeros((128, 128), np.float32)
    pf[0:64, 0:64] = p        # head A block
    pf[64:128, 64:128] = p    # head B block
    return np.ascontiguousarray(pf)


def make_core_inputs(x, qkv_w, qkv_b, proj_w, NB, T):
    NTOK = NB * T
    xTf = np.ascontiguousarray(
        np.asarray(x).reshape(NTOK, HIDDEN).astype(np.float32).T)
    cos_t, sin_t = _rope_tables(T)
    psgn = _psgn()
    in_maps = []
    for c in range(N_CORES):
        col = HD * H2 * c
        wq = qkv_w[:, col:col + 128]
        wk = qkv_w[:, HIDDEN + col:HIDDEN + col + 128]
        wv = qkv_w[:, 2 * HIDDEN + col:2 * HIDDEN + col + 128]
        wc = np.ascontiguousarray(
            np.concatenate([wq, wk, wv], axis=1).astype(np.float32))
        bq = qkv_b[col:col + 128]
        bk = qkv_b[HIDDEN + col:HIDDEN + col + 128]
        bv = qkv_b[2 * HIDDEN + col:2 * HIDDEN + col + 128]
        bcolc = np.ascontiguousarray(
            np.stack([bq, bk, bv], axis=1).astype(np.float32))
        pwc = np.ascontiguousarray(
            proj_w[col:col + 128, :].astype(np.float32))
        in_maps.append({
            "xT": xTf, "w": wc, "bcol": bcolc, "psgn": psgn, "pw": pwc,
            "cos_t": cos_t, "sin_t": sin_t,
        })
    return in_maps


_PROGRAM_CACHE = {}


def _get_program(NB, T):
    key = (NB, T)
    if key not in _PROGRAM_CACHE:
        _PROGRAM_CACHE[key] = build_program(NB, T)
    return _PROGRAM_CACHE[key]


def run(x, qkv_w, qkv_b, proj_w, proj_b, NB, T, trace=False):
    nc = _get_program(NB, T)
    in_maps = make_core_inputs(x, qkv_w, qkv_b, proj_w, NB, T)
    res = bass_utils.run_bass_kernel_spmd(
        nc, in_maps, core_ids=list(range(N_CORES)), trace=trace)
    acc = res.results[0]["y"].astype(np.float32).copy()
    for c in range(1, N_CORES):
        acc += res.results[c]["y"]
    out = (acc.reshape(NB, T, HIDDEN)
           + np.asarray(proj_b)[None, None, :].astype(np.float32))
    return out, res


def kernel(x, qkv_w, qkv_b, proj_w, proj_b):
    x = np.asarray(x)
    B, L, D = x.shape
    out, _ = run(x, np.asarray(qkv_w), np.asarray(qkv_b),
                 np.asarray(proj_w), np.asarray(proj_b), NB=B, T=L)
    return out.astype(np.float32)


# revision 13
# speedup vs baseline: 1.1771x; 1.0282x over previous
"""Trainium2 Bass kernel for nn_CausalSelfAttention (B=4, L=2048, D=1024, H=16).

Sharding: 2 heads per core (tensor parallel) x 8 cores, all batches on every
core.  Each core computes qkv for its 2 heads over all tokens, runs causal
attention, and produces a partial projection output y_c [NTOK, D] (its heads'
contribution).  The host sums the 8 partials and adds proj_b.

Key layout/perf choices vs the v0 kernel:
  * x is transposed on the HOST -> the kernel DMAs xT [d, tok] tiles straight
    to SBUF; no PE transposes / DVE casts for x at all.
  * qkv bias is added by the DVE PSUM->SBUF copy (tensor_scalar_add with a
    per-partition bias AP) instead of K=1 matmuls.
  * proj emits y in [tok, feat] layout (lhsT = O^T chunk, rhs = proj_w rows)
    so the host reduction needs no transpose.
  * emission is software-pipelined: the attention kb-loop of chunk i carries
    interleaved packets of chunk i+1's qkv matmuls and chunk i-1's proj,
    keeping the PE array continuously busy (the tensor engine clock ramps to
    full speed only after ~3us of uninterrupted work; measured matmuls run
    2x faster deep inside long busy runs).
  * KT/V residents are PER-CHUNK tiles (x2 batch parity) so pipelined rope
    writes never share a tile with in-flight attention reads.
  * O is normalized immediately after its last PV so its PSUM banks recycle
    with minimal stall; PSUM fits exactly: scores 2x2 banks, O 2x1, matmul
    scratch (qkv+proj, one pool) 2x1.
"""

import numpy as np

import concourse.bass as bass  # noqa: F401
import concourse.tile as tile
from concourse import mybir, bacc
from concourse import bass_utils
from concourse.masks import make_identity

f32 = mybir.dt.float32
f32r = mybir.dt.float32r
AL = mybir.AluOpType
AF = mybir.ActivationFunctionType


class _Bacc(bacc.Bacc):
    """Pin activations to the table set holding both ln and exp so ACT never
    thrashs ACT_TABLE_LOADs (~1.3us each)."""

    def insert_act_table_loads(self):
        import bass_rust as _bass_rust
        from concourse.hw_specs import get_activation_tables

        has_activation = any(
            isinstance(i, mybir.InstActivation)
            for bb in self.main_func.blocks
            for i in bb.instructions
        )
        if not has_activation:
            return
        tables = [
            (k, v if k == "natural_log_exp_and_others" else set())
            for k, v in get_activation_tables(self.m.arch).items()
        ]
        _bass_rust.insert_act_table_loads(self, tables)


HIDDEN = 1024
HEADS = 16
HD = 64
ROPE_BASE = 10000.0
N_CORES = 8
H2 = 2           # heads per core
F = 3 * H2 * HD  # 384 qkv feature columns per core
QCH = 512        # token chunk = attention q granule
DT = HIDDEN // 128  # 8 d tiles
KPC = QCH // 128    # k tiles per chunk (4)


def build_program(NB, T):
    """Build the per-core Bass program: NB batches of T tokens each."""
    assert T % QCH == 0
    NTOK = NB * T
    NCH = T // QCH   # q chunks per batch
    NITER = NB * NCH
    nc = _Bacc("TRN2", target_bir_lowering=False, debug=False,
               num_devices=N_CORES)

    xT = nc.dram_tensor("xT", [HIDDEN, NTOK], f32r, kind="ExternalInput").ap()
    w = nc.dram_tensor("w", [HIDDEN, F], f32r, kind="ExternalInput").ap()
    bcol = nc.dram_tensor("bcol", [128, 3], f32, kind="ExternalInput").ap()
    psgn = nc.dram_tensor("psgn", [128, 128], f32r, kind="ExternalInput").ap()
    pw = nc.dram_tensor("pw", [128, HIDDEN], f32r, kind="ExternalInput").ap()
    cos_t = nc.dram_tensor("cos_t", [128, T], f32, kind="ExternalInput").ap()
    sin_t = nc.dram_tensor("sin_t", [128, T], f32, kind="ExternalInput").ap()
    y = nc.dram_tensor("y", [NTOK, HIDDEN], f32, kind="ExternalOutput").ap()

    with tile.TileContext(nc) as tc:
        with tc.tile_pool(name="const", bufs=1) as constp, \
             tc.tile_pool(name="xload", bufs=3) as xp, \
             tc.tile_pool(name="rope", bufs=3) as ropep, \
             tc.tile_pool(name="qtcur", bufs=2) as qtp, \
             tc.tile_pool(name="pt", bufs=3) as ptp, \
             tc.tile_pool(name="ot", bufs=2) as otp, \
             tc.tile_pool(name="ysb", bufs=3) as yp, \
             tc.tile_pool(name="small", bufs=4) as smp, \
             tc.tile_pool(name="ps_s", bufs=2, space="PSUM") as ps_s_p, \
             tc.tile_pool(name="ps_o", bufs=2, space="PSUM") as ps_o_p, \
             tc.tile_pool(name="ps_m", bufs=2, space="PSUM") as ps_m_p:

            # ---- constants / residents ----
            ident_f = constp.tile([128, 128], f32)
            make_identity(nc, ident_f[:])
            ident = constp.tile([128, 128], f32r)
            nc.vector.tensor_copy(ident[:], ident_f[:])
            w_sb = constp.tile([128, DT * F], f32r)
            for dt in range(DT):
                nc.sync.dma_start(w_sb[:, dt * F:(dt + 1) * F],
                                  w[dt * 128:(dt + 1) * 128, :])
            bcol_sb = constp.tile([128, 3], f32)
            nc.sync.dma_start(bcol_sb[:], bcol[:])
            psgn_sb = constp.tile([128, 128], f32r)
            nc.sync.dma_start(psgn_sb[:], psgn[:])
            ones_f = constp.tile([128, 2 * KPC], f32)
            nc.gpsimd.memset(ones_f[:], 1.0)
            pw_sb = constp.tile([128, HIDDEN], f32r)
            nc.sync.dma_start(pw_sb[:], pw[:])
            cos_sb = constp.tile([128, T], f32)
            nc.sync.dma_start(cos_sb[:], cos_t[:])
            sin_sb = constp.tile([128, T], f32)
            nc.sync.dma_start(sin_sb[:], sin_t[:])

            # Per-(parity, chunk) KT / V resident tiles.  Separate tiles (not
            # column ranges of one resident) so pipelined rope/V writes of
            # chunk i+1 share no tile with chunk i's attention reads, and
            # batch parity keeps batch b+1's writes clear of batch b reads.
            KT_t = [[constp.tile([128, QCH], f32r, name=f"KT{p}_{qc}")
                     for qc in range(NCH)] for p in range(2)]
            V_t = [[constp.tile([128, KPC * 130], f32r, name=f"V{p}_{qc}")
                    for qc in range(NCH)] for p in range(2)]
            for p in range(2):
                for qc in range(NCH):
                    v4 = V_t[p][qc][:].rearrange("p (kt h c) -> p kt h c",
                                                 kt=KPC, h=2)
                    nc.gpsimd.tensor_copy(
                        v4[:, :, :, 64],
                        ones_f[:].rearrange("p (kt h) -> p kt h", kt=KPC))

            # per-iteration state passed between pipeline stages
            qt_tiles = {}    # it -> QT tile
            xa_tiles = {}    # it -> x chunk tile
            o_tiles = {}     # it -> (O[h] psum tiles, t0)
            ot_tiles = {}    # it -> (normalized O in SBUF, t0)

            def emit_xload(it):
                b, qc = divmod(it, NCH)
                t0 = b * T + qc * QCH
                xa = xp.tile([128, DT * QCH], f32r, tag="xa", name="xa")
                src = xT[:].rearrange("(dt p) n -> p dt n", p=128)
                nc.sync.dma_start(
                    xa[:].rearrange("p (dt n) -> p dt n", dt=DT),
                    src[:, :, t0:t0 + QCH])
                xa_tiles[it] = xa

            def qkv_packets(it):
                """Closures emitting chunk it's qkv phase (~1 PE-slot each).
                All PSUM allocation happens at emission time so ps_m pool
                rotation order matches the actual instruction order."""
                b, qc = divmod(it, NCH)
                Q0 = qc * QCH
                par = b % 2
                xa = xa_tiles.pop(it)
                pk = []
                QT_cur = qtp.tile([128, QCH], f32r, tag="qt", name="QT")
                qt_tiles[it] = QT_cur
                for f in range(3):  # 0=q, 1=k, 2=v
                    cell = {}

                    def mm2(f=f, dt0=0, cell=cell, xa=xa):
                        if dt0 == 0:
                            cell["ps"] = ps_m_p.tile([128, QCH], f32,
                                                     tag="m", name="ps_f")
                        for dt in (dt0, dt0 + 1):
                            nc.tensor.matmul(
                                cell["ps"][:],
                                w_sb[:, dt * F + f * 128:
                                     dt * F + (f + 1) * 128],
                                xa[:, dt * QCH:dt * QCH + QCH],
                                start=(dt == 0), stop=(dt == DT - 1))
                    for dt0 in range(0, DT, 2):
                        pk.append(lambda f=f, dt0=dt0, cell=cell, xa=xa:
                                  mm2(f, dt0, cell, xa))
                    raw = ropep.tile([128, QCH], f32r, tag="raw", name="raw")

                    def cp(raw=raw, cell=cell, f=f):
                        nc.vector.tensor_scalar_add(
                            raw[:], cell["ps"][:], bcol_sb[:, f:f + 1])
                    pk.append(cp)
                    if f < 2:
                        def rot(cell=cell, raw=raw):
                            cell["rot"] = ps_m_p.tile([128, QCH], f32,
                                                      tag="m", name="ps_rot")
                            nc.tensor.matmul(cell["rot"][:], psgn_sb[:],
                                             raw[:], start=True, stop=True)
                        pk.append(rot)
                        t1 = ropep.tile([128, QCH], f32, tag="t1", name="t1")
                        t2 = ropep.tile([128, QCH], f32, tag="t2", name="t2")
                        dst = (QT_cur[:] if f == 0 else KT_t[par][qc][:])

                        def rmul(t1=t1, t2=t2, raw=raw, cell=cell, Q0=Q0):
                            nc.vector.tensor_tensor(
                                t1[:], raw[:], cos_sb[:, Q0:Q0 + QCH], AL.mult)
                            nc.vector.tensor_tensor(
                                t2[:], cell["rot"][:], sin_sb[:, Q0:Q0 + QCH],
                                AL.mult)
                        pk.append(rmul)

                        def radd(dst=dst, t1=t1, t2=t2):
                            nc.vector.tensor_tensor(dst, t1[:], t2[:], AL.add)
                        pk.append(radd)
                    else:
                        for tt in range(4):
                            def vtr(tt=tt, raw=raw, par=par, qc=qc):
                                ps_vt = ps_m_p.tile([128, 128], f32r, tag="m",
                                                    name="ps_vt")
                                nc.tensor.transpose(
                                    ps_vt[:],
                                    raw[:, tt * 128:(tt + 1) * 128], ident[:])
                                nc.vector.tensor_copy(
                                    V_t[par][qc][:].rearrange(
                                        "p (kt h c) -> p kt h c",
                                        kt=KPC, h=2)[:, tt, :, 0:64],
                                    ps_vt[:].rearrange("p (h j) -> p h j",
                                                       h=2))
                            pk.append(vtr)
                return pk

            def attn_steps(it):
                """Per-kb closures for chunk it's attention; O accumulates in
                PSUM across all kb."""
                b, qc = divmod(it, NCH)
                Q0 = qc * QCH
                par = b % 2
                t0 = b * T + Q0
                nkb = (Q0 + QCH) // 128
                QT_cur = qt_tiles.pop(it)
                O = [ps_o_p.tile([65, QCH], f32, tag="o", name=f"O{h}")
                     for h in range(2)]
                o_tiles[it] = (O, t0)
                sc_tiles = {}
                pt_tiles = {}

                def emit_S(kb):
                    qstart = max(0, 128 * kb - Q0)
                    ps_sc = ps_s_p.tile([128, 2 * QCH], f32, tag="s",
                                        name="ps_sc")
                    sc_tiles[kb] = ps_sc
                    kt = KT_t[par][kb // KPC]
                    kl = kb % KPC
                    for h in range(2):
                        hp = slice(64 * h, 64 * h + 64)
                        nc.tensor.matmul(
                            ps_sc[:, QCH * h + qstart:QCH * h + QCH],
                            kt[hp, kl * 128:(kl + 1) * 128],
                            QT_cur[hp, qstart:QCH],
                            start=True, stop=True)

                def step(kb):
                    if kb == 0:
                        emit_S(0)
                    qstart = max(0, 128 * kb - Q0)
                    ps_sc = sc_tiles.pop(kb)
                    pt = ptp.tile([128, 2 * QCH], f32r, tag="pt", name="pt")
                    pt_tiles[kb] = pt
                    sc4 = ps_sc[:].rearrange("p (h q) -> p h q", h=2)
                    pt4 = pt[:].rearrange("p (h q) -> p h q", h=2)
                    nc.scalar.activation(pt4[:, :, qstart:QCH],
                                         sc4[:, :, qstart:QCH],
                                         AF.Exp, bias=0.0, scale=0.125)
                    if 128 * kb >= Q0:
                        ds = 128 * kb - Q0
                        for h in range(2):
                            nc.gpsimd.affine_select(
                                out=pt[:, QCH * h + ds:QCH * h + ds + 128],
                                in_=pt[:, QCH * h + ds:QCH * h + ds + 128],
                                pattern=[[1, 128]], compare_op=AL.is_ge,
                                fill=0.0, base=0, channel_multiplier=-1)
                    if kb + 1 < nkb:
                        emit_S(kb + 1)

                def pv(kb):
                    qstart = max(0, 128 * kb - Q0)
                    pt = pt_tiles.pop(kb)
                    vt = V_t[par][kb // KPC]
                    kl = kb % KPC
                    for h in range(2):
                        nc.tensor.matmul(
                            O[h][:, qstart:QCH],
                            vt[:, 130 * kl + 65 * h:130 * kl + 65 * h + 65],
                            pt[:, QCH * h + qstart:QCH * h + QCH],
                            start=(kb == 0), stop=(kb == nkb - 1))
                return [(lambda kb=kb: step(kb), lambda kb=kb: pv(kb))
                        for kb in range(nkb)]

            def emit_norm(it):
                """Normalize O(it) -> ot_full; emitted right after the last
                PV so O's PSUM banks recycle as early as possible."""
                O, t0v = o_tiles.pop(it)
                ot_full = otp.tile([128, QCH], f32r, tag="ot", name="ot_full")
                ot_tiles[it] = (ot_full, t0v)
                for h in range(2):
                    lnv = smp.tile([1, QCH], f32, tag="ln", name="lnv")
                    nc.scalar.activation(lnv[:], O[h][64:65, :], AF.Ln)
                    rs = smp.tile([1, QCH], f32, tag="rs", name="rs")
                    nc.scalar.activation(rs[:], lnv[:], AF.Exp,
                                         bias=0.0, scale=-1.0)
                    rsb = smp.tile([64, QCH], f32, tag="rsb", name="rsb")
                    nc.gpsimd.partition_broadcast(rsb[:], rs[:])
                    nc.vector.tensor_tensor(ot_full[64 * h:64 * h + 64, :],
                                            O[h][0:64, :], rsb[:], AL.mult)

            # engines for the 8 proj PSUM->SBUF copies (GpSimd cannot read
            # PSUM): mostly DVE, keeping ACT free for the exp stream
            _cp_eng = ["v", "v", "a", "v", "v", "v", "a", "v"]

            def post_packets(it):
                """Project normalized O(it) to y — list of closures."""
                ot_full, t0v = ot_tiles.pop(it)
                pk = []
                for oi in range(8):
                    def proj(oi=oi, ot_full=ot_full, t0v=t0v):
                        tt, half = divmod(oi, 2)
                        ps_y = ps_m_p.tile([128, QCH], f32, tag="m",
                                           name="ps_y")
                        nc.tensor.matmul(
                            ps_y[:], ot_full[:, tt * 128:(tt + 1) * 128],
                            pw_sb[:, half * QCH:(half + 1) * QCH],
                            start=True, stop=True)
                        ysb = yp.tile([128, QCH], f32, tag="y", name="ysb")
                        if _cp_eng[oi] == "v":
                            nc.vector.tensor_copy(ysb[:], ps_y[:])
                        else:
                            nc.scalar.copy(ysb[:], ps_y[:])
                        nc.sync.dma_start(
                            y[t0v + tt * 128:t0v + (tt + 1) * 128,
                              half * QCH:(half + 1) * QCH], ysb[:])
                    pk.append(proj)
                return pk

            # ---------------- pipelined emission ----------------
            emit_xload(0)
            for p in qkv_packets(0):
                p()
            for it in range(NITER):
                if it + 1 < NITER:
                    emit_xload(it + 1)
                extras = []
                if it >= 1:
                    extras += post_packets(it - 1)
                if it + 1 < NITER:
                    extras += qkv_packets(it + 1)
                steps = attn_steps(it)
                nkb = len(steps)
                ei = 0
                for kb, (st, pv) in enumerate(steps):
                    st()
                    want = (kb + 1) * len(extras) // nkb
                    while ei < want:
                        extras[ei]()
                        ei += 1
                    pv()
                emit_norm(it)
                while ei < len(extras):
                    extras[ei]()
                    ei += 1
            for p in post_packets(NITER - 1):
                p()
    nc.compile()
    return nc


# ---------------------------------------------------------------- host side

def _rope_tables(T):
    inv_freq = 1.0 / (ROPE_BASE ** (np.arange(0, HD, 2, dtype=np.float64) / HD))
    pos = np.arange(T, dtype=np.float64)
    ang = np.outer(pos, inv_freq)          # [T, 32]
    cos = np.cos(ang).astype(np.float32)   # [T, 32]
    sin = np.sin(ang).astype(np.float32)
    jm32 = np.arange(128) % 32
    cos_t = np.ascontiguousarray(cos[:, jm32].T)   # [128, T]
    sin_t = np.ascontiguousarray(sin[:, jm32].T)
    return cos_t, sin_t


def _psgn():
    p = np.zeros((HD, HD), np.float32)
    for i in range(32):
        p[i + 32, i] = -1.0   # out dim i (<32) = -in dim i+32
        p[i, i + 32] = 1.0    # out dim i+32   = +in dim i
    pf = np.zeros((128, 128), np.float32)
    pf[0:64, 0:64] = p        # head A block
    pf[64:128, 64:128] = p    # head B block
    return np.ascontiguousarray(pf)


def make_core_inputs(x, qkv_w, qkv_b, proj_w, NB, T):
    NTOK = NB * T
    xTf = np.ascontiguousarray(
        np.asarray(x).reshape(NTOK, HIDDEN).astype(np.float32).T)
    cos_t, sin_t = _rope_tables(T)
    psgn = _psgn()
    in_maps = []
    for c in range(N_CORES):
        col = HD * H2 * c
        wq = qkv_w[:, col:col + 128]
        wk = qkv_w[:, HIDDEN + col:HIDDEN + col + 128]
        wv = qkv_w[:, 2 * HIDDEN + col:2 * HIDDEN + col + 128]
        wc = np.ascontiguousarray(
            np.concatenate([wq, wk, wv], axis=1).astype(np.float32))
        bq = qkv_b[col:col + 128]
        bk = qkv_b[HIDDEN + col:HIDDEN + col + 128]
        bv = qkv_b[2 * HIDDEN + col:2 * HIDDEN + col + 128]
        bcolc = np.ascontiguousarray(
            np.stack([bq, bk, bv], axis=1).astype(np.float32))
        pwc = np.ascontiguousarray(
            proj_w[col:col + 128, :].astype(np.float32))
        in_maps.append({
            "xT": xTf, "w": wc, "bcol": bcolc, "psgn": psgn, "pw": pwc,
            "cos_t": cos_t, "sin_t": sin_t,
        })
    return in_maps


_PROGRAM_CACHE = {}


def _get_program(NB, T):
    key = (NB, T)
    if key not in _PROGRAM_CACHE:
        _PROGRAM_CACHE[key] = build_program(NB, T)
    return _PROGRAM_CACHE[key]


def run(x, qkv_w, qkv_b, proj_w, proj_b, NB, T, trace=False):
    nc = _get_program(NB, T)
    in_maps = make_core_inputs(x, qkv_w, qkv_b, proj_w, NB, T)
    res = bass_utils.run_bass_kernel_spmd(
        nc, in_maps, core_ids=list(range(N_CORES)), trace=trace)
    acc = res.results[0]["y"].astype(np.float32).copy()
    for c in range(1, N_CORES):
        acc += res.results[c]["y"]
    out = (acc.reshape(NB, T, HIDDEN)
           + np.asarray(proj_b)[None, None, :].astype(np.float32))
    return out, res


def kernel(x, qkv_w, qkv_b, proj_w, proj_b):
    x = np.asarray(x)
    B, L, D = x.shape
    out, _ = run(x, np.asarray(qkv_w), np.asarray(qkv_b),
                 np.asarray(proj_w), np.asarray(proj_b), NB=B, T=L)
    return out.astype(np.float32)


# revision 17
# speedup vs baseline: 1.3468x; 1.1442x over previous
"""Trainium2 Bass kernel for nn_CausalSelfAttention (B=4, L=2048, D=1024, H=16).

Sharding: 2 heads per core (tensor parallel) x 8 cores, all batches on every
core.  Each core computes qkv for its 2 heads over all tokens, runs causal
attention, and produces a partial projection output y_c [NTOK, D] (its heads'
contribution).  The host sums the 8 partials and adds proj_b.

Key layout/perf choices vs the v0 kernel:
  * x is transposed on the HOST -> the kernel DMAs xT [d, tok] tiles straight
    to SBUF; no PE transposes / DVE casts for x at all.
  * qkv bias is added by the DVE PSUM->SBUF copy (tensor_scalar_add with a
    per-partition bias AP) instead of K=1 matmuls.
  * proj emits y in [tok, feat] layout (lhsT = O^T chunk, rhs = proj_w rows)
    so the host reduction needs no transpose.
  * emission is software-pipelined: the attention kb-loop of chunk i carries
    interleaved packets of chunk i+1's qkv matmuls and chunk i-1's proj,
    keeping the PE array continuously busy (the tensor engine clock ramps to
    full speed only after ~3us of uninterrupted work; measured matmuls run
    2x faster deep inside long busy runs).
  * KT/V residents are PER-CHUNK tiles (x2 batch parity) so pipelined rope
    writes never share a tile with in-flight attention reads.
  * O is normalized immediately after its last PV so its PSUM banks recycle
    with minimal stall; PSUM fits exactly: scores 2x2 banks, O 2x1, matmul
    scratch (qkv+proj, one pool) 2x1.
"""

import numpy as np

import concourse.bass as bass  # noqa: F401
import concourse.tile as tile
from concourse import mybir, bacc
from concourse import bass_utils
from concourse.masks import make_identity

f32 = mybir.dt.float32
f32r = mybir.dt.float32r
AL = mybir.AluOpType
AF = mybir.ActivationFunctionType


class _Bacc(bacc.Bacc):
    """Pin activations to the table set holding both ln and exp so ACT never
    thrashs ACT_TABLE_LOADs (~1.3us each)."""

    def insert_act_table_loads(self):
        import bass_rust as _bass_rust
        from concourse.hw_specs import get_activation_tables

        has_activation = any(
            isinstance(i, mybir.InstActivation)
            for bb in self.main_func.blocks
            for i in bb.instructions
        )
        if not has_activation:
            return
        tables = [
            (k, v if k == "natural_log_exp_and_others" else set())
            for k, v in get_activation_tables(self.m.arch).items()
        ]
        _bass_rust.insert_act_table_loads(self, tables)


HIDDEN = 1024
HEADS = 16
HD = 64
ROPE_BASE = 10000.0
N_CORES = 8
H2 = 2           # heads per core
F = 3 * H2 * HD  # 384 qkv feature columns per core
QCH = 512        # token chunk = attention q granule
DT = HIDDEN // 128  # 8 d tiles
KPC = QCH // 128    # k tiles per chunk (4)


def build_program(NB, T):
    """Build the per-core Bass program: NB batches of T tokens each."""
    assert T % QCH == 0
    NTOK = NB * T
    NCH = T // QCH   # q chunks per batch
    NITER = NB * NCH
    nc = _Bacc("TRN2", target_bir_lowering=False, debug=False,
               num_devices=N_CORES)

    xT = nc.dram_tensor("xT", [HIDDEN, NTOK], f32r, kind="ExternalInput").ap()
    w = nc.dram_tensor("w", [HIDDEN, F], f32r, kind="ExternalInput").ap()
    bcol = nc.dram_tensor("bcol", [128, 3], f32, kind="ExternalInput").ap()
    psgn = nc.dram_tensor("psgn", [128, 128], f32r, kind="ExternalInput").ap()
    pw = nc.dram_tensor("pw", [128, HIDDEN], f32r, kind="ExternalInput").ap()
    cos_t = nc.dram_tensor("cos_t", [128, T], f32, kind="ExternalInput").ap()
    sin_t = nc.dram_tensor("sin_t", [128, T], f32, kind="ExternalInput").ap()
    y = nc.dram_tensor("y", [NTOK, HIDDEN], f32, kind="ExternalOutput").ap()

    with tile.TileContext(nc) as tc:
        with tc.tile_pool(name="const", bufs=1) as constp, \
             tc.tile_pool(name="xload", bufs=3) as xp, \
             tc.tile_pool(name="rope", bufs=3) as ropep, \
             tc.tile_pool(name="qtcur", bufs=2) as qtp, \
             tc.tile_pool(name="pt", bufs=3) as ptp, \
             tc.tile_pool(name="ot", bufs=2) as otp, \
             tc.tile_pool(name="ysb", bufs=3) as yp, \
             tc.tile_pool(name="small", bufs=4) as smp, \
             tc.tile_pool(name="ps_s", bufs=2, space="PSUM") as ps_s_p, \
             tc.tile_pool(name="ps_o", bufs=2, space="PSUM") as ps_o_p, \
             tc.tile_pool(name="ps_m", bufs=2, space="PSUM") as ps_m_p:

            # ---- constants / residents ----
            ident_f = constp.tile([128, 128], f32)
            make_identity(nc, ident_f[:])
            ident = constp.tile([128, 128], f32r)
            nc.vector.tensor_copy(ident[:], ident_f[:])
            w_sb = constp.tile([128, DT * F], f32r)
            for dt in range(DT):
                nc.sync.dma_start(w_sb[:, dt * F:(dt + 1) * F],
                                  w[dt * 128:(dt + 1) * 128, :])
            bcol_sb = constp.tile([128, 3], f32)
            nc.sync.dma_start(bcol_sb[:], bcol[:])
            psgn_sb = constp.tile([128, 128], f32r)
            nc.sync.dma_start(psgn_sb[:], psgn[:])
            ones_f = constp.tile([128, 2 * KPC], f32)
            nc.gpsimd.memset(ones_f[:], 1.0)
            pw_sb = constp.tile([128, HIDDEN], f32r)
            nc.sync.dma_start(pw_sb[:], pw[:])
            cos_sb = constp.tile([128, T], f32)
            nc.sync.dma_start(cos_sb[:], cos_t[:])
            sin_sb = constp.tile([128, T], f32)
            nc.sync.dma_start(sin_sb[:], sin_t[:])

            # Per-(parity, chunk) KT / V resident tiles.  Separate tiles (not
            # column ranges of one resident) so pipelined rope/V writes of
            # chunk i+1 share no tile with chunk i's attention reads, and
            # batch parity keeps batch b+1's writes clear of batch b reads.
            KT_t = [[constp.tile([128, QCH], f32r, name=f"KT{p}_{qc}")
                     for qc in range(NCH)] for p in range(2)]
            V_t = [[constp.tile([128, KPC * 130], f32r, name=f"V{p}_{qc}")
                    for qc in range(NCH)] for p in range(2)]
            for p in range(2):
                for qc in range(NCH):
                    v4 = V_t[p][qc][:].rearrange("p (kt h c) -> p kt h c",
                                                 kt=KPC, h=2)
                    nc.gpsimd.tensor_copy(
                        v4[:, :, :, 64],
                        ones_f[:].rearrange("p (kt h) -> p kt h", kt=KPC))

            # per-iteration state passed between pipeline stages
            qt_tiles = {}    # it -> QT tile
            xa_tiles = {}    # it -> x chunk tile
            o_tiles = {}     # it -> (O[h] psum tiles, t0)
            ot_tiles = {}    # it -> (normalized O in SBUF, t0)

            def emit_xload(it):
                b, qc = divmod(it, NCH)
                t0 = b * T + qc * QCH
                xa = xp.tile([128, DT * QCH], f32r, tag="xa", name="xa")
                src = xT[:].rearrange("(dt p) n -> p dt n", p=128)
                nc.sync.dma_start(
                    xa[:].rearrange("p (dt n) -> p dt n", dt=DT),
                    src[:, :, t0:t0 + QCH])
                xa_tiles[it] = xa

            def qkv_packets(it):
                """Closures emitting chunk it's qkv phase (~1 PE-slot each).
                All PSUM allocation happens at emission time so ps_m pool
                rotation order matches the actual instruction order."""
                b, qc = divmod(it, NCH)
                Q0 = qc * QCH
                par = b % 2
                xa = xa_tiles.pop(it)
                pk = []
                QT_cur = qtp.tile([128, QCH], f32r, tag="qt", name="QT")
                qt_tiles[it] = QT_cur
                for f in (1, 0, 2):  # k first: its rope output unblocks the
                                     # next chunk's diagonal S earliest
                    cell = {}

                    def mm2(f=f, dt0=0, cell=cell, xa=xa):
                        if dt0 == 0:
                            cell["ps"] = ps_m_p.tile([128, QCH], f32,
                                                     tag="m", name="ps_f")
                        for dt in (dt0, dt0 + 1):
                            nc.tensor.matmul(
                                cell["ps"][:],
                                w_sb[:, dt * F + f * 128:
                                     dt * F + (f + 1) * 128],
                                xa[:, dt * QCH:dt * QCH + QCH],
                                start=(dt == 0), stop=(dt == DT - 1))
                    for dt0 in range(0, DT, 2):
                        pk.append(lambda f=f, dt0=dt0, cell=cell, xa=xa:
                                  mm2(f, dt0, cell, xa))
                    raw = ropep.tile([128, QCH], f32r, tag="raw", name="raw")

                    def cp(raw=raw, cell=cell, f=f):
                        nc.vector.tensor_scalar_add(
                            raw[:], cell["ps"][:], bcol_sb[:, f:f + 1])
                    pk.append(cp)
                    if f < 2:
                        def rot(cell=cell, raw=raw):
                            cell["rot"] = ps_m_p.tile([128, QCH], f32,
                                                      tag="m", name="ps_rot")
                            nc.tensor.matmul(cell["rot"][:], psgn_sb[:],
                                             raw[:], start=True, stop=True)
                        pk.append(rot)
                        t1 = ropep.tile([128, QCH], f32, tag="t1", name="t1")
                        t2 = ropep.tile([128, QCH], f32, tag="t2", name="t2")
                        dst = (QT_cur[:] if f == 0 else KT_t[par][qc][:])

                        def rmul(t1=t1, t2=t2, raw=raw, cell=cell, Q0=Q0):
                            nc.vector.tensor_tensor(
                                t1[:], raw[:], cos_sb[:, Q0:Q0 + QCH], AL.mult)
                            nc.vector.tensor_tensor(
                                t2[:], cell["rot"][:], sin_sb[:, Q0:Q0 + QCH],
                                AL.mult)
                        pk.append(rmul)

                        def radd(dst=dst, t1=t1, t2=t2):
                            nc.vector.tensor_tensor(dst, t1[:], t2[:], AL.add)
                        pk.append(radd)
                    else:
                        vcell = {}

                        def vtr(tt, raw=raw, vcell=vcell):
                            if tt == 0:
                                vcell["ps"] = ps_m_p.tile(
                                    [128, QCH], f32r, tag="m", name="ps_v4")
                            nc.tensor.transpose(
                                vcell["ps"][:, tt * 128:(tt + 1) * 128],
                                raw[:, tt * 128:(tt + 1) * 128], ident[:])
                        for tt in range(4):
                            pk.append(lambda tt=tt: vtr(tt))

                        def vcast(vcell=vcell, par=par, qc=qc):
                            nc.vector.tensor_copy(
                                V_t[par][qc][:].rearrange(
                                    "p (kt h c) -> p kt h c",
                                    kt=KPC, h=2)[:, :, :, 0:64],
                                vcell["ps"][:].rearrange(
                                    "p (kt h j) -> p kt h j", kt=KPC, h=2))
                        pk.append(vcast)
                return pk

            def attn_steps(it):
                """Per-kb closures for chunk it's attention; O accumulates in
                PSUM across all kb."""
                b, qc = divmod(it, NCH)
                Q0 = qc * QCH
                par = b % 2
                t0 = b * T + Q0
                nkb = (Q0 + QCH) // 128
                QT_cur = qt_tiles.pop(it)
                O = [ps_o_p.tile([65, QCH], f32, tag="o", name=f"O{h}")
                     for h in range(2)]
                o_tiles[it] = (O, t0)
                sc_tiles = {}
                pt_tiles = {}

                def emit_S(kb):
                    qstart = max(0, 128 * kb - Q0)
                    ps_sc = ps_s_p.tile([128, 2 * QCH], f32, tag="s",
                                        name="ps_sc")
                    sc_tiles[kb] = ps_sc
                    kt = KT_t[par][kb // KPC]
                    kl = kb % KPC
                    for h in range(2):
                        hp = slice(64 * h, 64 * h + 64)
                        nc.tensor.matmul(
                            ps_sc[:, QCH * h + qstart:QCH * h + QCH],
                            kt[hp, kl * 128:(kl + 1) * 128],
                            QT_cur[hp, qstart:QCH],
                            start=True, stop=True)

                def step(kb):
                    if kb == 0:
                        emit_S(0)
                    qstart = max(0, 128 * kb - Q0)
                    ps_sc = sc_tiles.pop(kb)
                    pt = ptp.tile([128, 2 * QCH], f32r, tag="pt", name="pt")
                    pt_tiles[kb] = pt
                    sc4 = ps_sc[:].rearrange("p (h q) -> p h q", h=2)
                    pt4 = pt[:].rearrange("p (h q) -> p h q", h=2)
                    nc.scalar.activation(pt4[:, :, qstart:QCH],
                                         sc4[:, :, qstart:QCH],
                                         AF.Exp, bias=0.0, scale=0.125)
                    if 128 * kb >= Q0:
                        ds = 128 * kb - Q0
                        for h in range(2):
                            nc.gpsimd.affine_select(
                                out=pt[:, QCH * h + ds:QCH * h + ds + 128],
                                in_=pt[:, QCH * h + ds:QCH * h + ds + 128],
                                pattern=[[1, 128]], compare_op=AL.is_ge,
                                fill=0.0, base=0, channel_multiplier=-1)
                    if kb + 1 < nkb:
                        emit_S(kb + 1)

                def pv(kb):
                    qstart = max(0, 128 * kb - Q0)
                    pt = pt_tiles.pop(kb)
                    vt = V_t[par][kb // KPC]
                    kl = kb % KPC
                    for h in range(2):
                        nc.tensor.matmul(
                            O[h][:, qstart:QCH],
                            vt[:, 130 * kl + 65 * h:130 * kl + 65 * h + 65],
                            pt[:, QCH * h + qstart:QCH * h + QCH],
                            start=(kb == 0), stop=(kb == nkb - 1))
                return [(lambda kb=kb: step(kb), lambda kb=kb: pv(kb))
                        for kb in range(nkb)]

            def emit_norm(it):
                """Normalize O(it) -> ot_full; emitted right after the last
                PV so O's PSUM banks recycle as early as possible."""
                O, t0v = o_tiles.pop(it)
                ot_full = otp.tile([128, QCH], f32r, tag="ot", name="ot_full")
                ot_tiles[it] = (ot_full, t0v)
                for h in range(2):
                    lnv = smp.tile([1, QCH], f32, tag="ln", name="lnv")
                    nc.scalar.activation(lnv[:], O[h][64:65, :], AF.Ln)
                    rs = smp.tile([1, QCH], f32, tag="rs", name="rs")
                    nc.scalar.activation(rs[:], lnv[:], AF.Exp,
                                         bias=0.0, scale=-1.0)
                    rsb = smp.tile([64, QCH], f32, tag="rsb", name="rsb")
                    nc.gpsimd.partition_broadcast(rsb[:], rs[:])
                    nc.vector.tensor_tensor(ot_full[64 * h:64 * h + 64, :],
                                            O[h][0:64, :], rsb[:], AL.mult)

            # engines for the 8 proj PSUM->SBUF copies (GpSimd cannot read
            # PSUM): mostly DVE, keeping ACT free for the exp stream
            _cp_eng = ["v", "v", "a", "v", "v", "v", "a", "v"]

            def post_packets(it):
                """Project normalized O(it) to y — list of closures."""
                ot_full, t0v = ot_tiles.pop(it)
                pk = []
                for oi in range(8):
                    def proj(oi=oi, ot_full=ot_full, t0v=t0v):
                        tt, half = divmod(oi, 2)
                        ps_y = ps_m_p.tile([128, QCH], f32, tag="m",
                                           name="ps_y")
                        nc.tensor.matmul(
                            ps_y[:], ot_full[:, tt * 128:(tt + 1) * 128],
                            pw_sb[:, half * QCH:(half + 1) * QCH],
                            start=True, stop=True)
                        ysb = yp.tile([128, QCH], f32, tag="y", name="ysb")
                        if _cp_eng[oi] == "v":
                            nc.vector.tensor_copy(ysb[:], ps_y[:])
                        else:
                            nc.scalar.copy(ysb[:], ps_y[:])
                        nc.sync.dma_start(
                            y[t0v + tt * 128:t0v + (tt + 1) * 128,
                              half * QCH:(half + 1) * QCH], ysb[:])
                    pk.append(proj)
                return pk

            # ---------------- pipelined emission ----------------
            emit_xload(0)
            for p in qkv_packets(0):
                p()
            for it in range(NITER):
                if it + 1 < NITER:
                    emit_xload(it + 1)
                # Interleave qkv(i+1) 2:1 ahead of proj(i-1), with the first
                # proj delayed ~3 packets: the norm chain producing ot_full
                # spans ACT->GPS->DVE and needs a few microseconds before the
                # first proj matmul can issue without stalling the PE.
                qk = qkv_packets(it + 1) if it + 1 < NITER else []
                pj = post_packets(it - 1) if it >= 1 else []
                extras = []
                qi = pi = 0
                while qi < len(qk) or pi < len(pj):
                    for _ in range(2):
                        if qi < len(qk):
                            extras.append(qk[qi])
                            qi += 1
                    if pi < len(pj) and (len(extras) >= 3 or qi >= len(qk)):
                        extras.append(pj[pi])
                        pi += 1
                steps = attn_steps(it)
                nkb = len(steps)
                ei = 0
                for kb, (st, pv) in enumerate(steps):
                    st()
                    want = (kb + 1) * len(extras) // nkb
                    while ei < want:
                        extras[ei]()
                        ei += 1
                    pv()
                emit_norm(it)
                while ei < len(extras):
                    extras[ei]()
                    ei += 1
            for p in post_packets(NITER - 1):
                p()
    nc.compile()
    return nc


# ---------------------------------------------------------------- host side

def _rope_tables(T):
    inv_freq = 1.0 / (ROPE_BASE ** (np.arange(0, HD, 2, dtype=np.float64) / HD))
    pos = np.arange(T, dtype=np.float64)
    ang = np.outer(pos, inv_freq)          # [T, 32]
    cos = np.cos(ang).astype(np.float32)   # [T, 32]
    sin = np.sin(ang).astype(np.float32)
    jm32 = np.arange(128) % 32
    cos_t = np.ascontiguousarray(cos[:, jm32].T)   # [128, T]
    sin_t = np.ascontiguousarray(sin[:, jm32].T)
    return cos_t, sin_t


def _psgn():
    p = np.zeros((HD, HD), np.float32)
    for i in range(32):
        p[i + 32, i] = -1.0   # out dim i (<32) = -in dim i+32
        p[i, i + 32] = 1.0    # out dim i+32   = +in dim i
    pf = np.zeros((128, 128), np.float32)
    pf[0:64, 0:64] = p        # head A block
    pf[64:128, 64:128] = p    # head B block
    return np.ascontiguousarray(pf)


def make_core_inputs(x, qkv_w, qkv_b, proj_w, NB, T):
    NTOK = NB * T
    xTf = np.ascontiguousarray(
        np.asarray(x).reshape(NTOK, HIDDEN).astype(np.float32).T)
    cos_t, sin_t = _rope_tables(T)
    psgn = _psgn()
    in_maps = []
    for c in range(N_CORES):
        col = HD * H2 * c
        wq = qkv_w[:, col:col + 128]
        wk = qkv_w[:, HIDDEN + col:HIDDEN + col + 128]
        wv = qkv_w[:, 2 * HIDDEN + col:2 * HIDDEN + col + 128]
        wc = np.ascontiguousarray(
            np.concatenate([wq, wk, wv], axis=1).astype(np.float32))
        bq = qkv_b[col:col + 128]
        bk = qkv_b[HIDDEN + col:HIDDEN + col + 128]
        bv = qkv_b[2 * HIDDEN + col:2 * HIDDEN + col + 128]
        bcolc = np.ascontiguousarray(
            np.stack([bq, bk, bv], axis=1).astype(np.float32))
        pwc = np.ascontiguousarray(
            proj_w[col:col + 128, :].astype(np.float32))
        in_maps.append({
            "xT": xTf, "w": wc, "bcol": bcolc, "psgn": psgn, "pw": pwc,
            "cos_t": cos_t, "sin_t": sin_t,
        })
    return in_maps


_PROGRAM_CACHE = {}


def _get_program(NB, T):
    key = (NB, T)
    if key not in _PROGRAM_CACHE:
        _PROGRAM_CACHE[key] = build_program(NB, T)
    return _PROGRAM_CACHE[key]


def run(x, qkv_w, qkv_b, proj_w, proj_b, NB, T, trace=False):
    nc = _get_program(NB, T)
    in_maps = make_core_inputs(x, qkv_w, qkv_b, proj_w, NB, T)
    res = bass_utils.run_bass_kernel_spmd(
        nc, in_maps, core_ids=list(range(N_CORES)), trace=trace)
    acc = res.results[0]["y"].astype(np.float32).copy()
    for c in range(1, N_CORES):
        acc += res.results[c]["y"]
    out = (acc.reshape(NB, T, HIDDEN)
           + np.asarray(proj_b)[None, None, :].astype(np.float32))
    return out, res


def kernel(x, qkv_w, qkv_b, proj_w, proj_b):
    x = np.asarray(x)
    B, L, D = x.shape
    out, _ = run(x, np.asarray(qkv_w), np.asarray(qkv_b),
                 np.asarray(proj_w), np.asarray(proj_b), NB=B, T=L)
    return out.astype(np.float32)
